# revision 1
# baseline (speedup 1.0000x reference)
"""Trainium2 Bass kernel for BioBERT-ARG-GNN (gated pooling + 2-layer GCN + MLP head).

Strategy: pure data parallel over batch B=64 across 8 NeuronCores (8
graphs per core).  Host precomputes index-derived structures (one-hot
pooling matrix P' with 1/cnt and D^-1/2 folded in, normalized adjacency
\hat A = D^-1/2 (A+I) D^-1/2) and ships them bf16 together with a bf16
TRANSPOSED copy of last_hidden (lhT, [hidden, tokens]) in ONE mega-tensor
per graph.  The transposed layout lets BOTH the gate logits and the W1
projection run on the PE with the contraction over the hidden dim:

    yT[gh, t] = sum_hc W1c^T  @ lhT_c    (6 matmuls, free=512)
    lg[:, t]  = sum_hc wrbc^T @ lhT_c    (6 matmuls, free=512; stationary =
                                          wr chunk broadcast to 128 columns,
                                          so lg is born partition-broadcast)

sigmoid(lg) on ACT -> [128, 512] gates; DVE multiplies them into yT while
casting to bf16; ONE DMA-XBAR transpose per graph PAIR turns ygT into
token-major y chunks; pooling then contracts tokens directly:
t1 = P'^T (g*y) = pool(gated lh) @ W1 — no nf materialization and no PE
transposes.  GCN layers use \hat A as the stationary operand; the FC head
is batched over all 8 graphs with zero transposes (cls ships pre-
transposed; the [2, BL] output is transposed back on the host).

Scheduling: a single software pipeline where step s issues graph s's
projection matmuls (paced by that graph's DMA) while pooling + the
five GCN stages for earlier graph pairs fill the PE slack.  DMA rings:
sync HWDGE carries meg 0,2,4,5,6,7; ACT HWDGE carries the phase-A
consts, meg 1,3, the late consts, and all XBAR transposes (kept free of
bulk traffic so transposes clear in data-ready order; the sequencers
park blocked instructions, so ring order must match data-ready order).
All matmuls are bf16 (PSUM f32): measured DVFS throttling caps the PE at
~55% utilization, which makes bf16 strictly better than fp32/f32r
(4x/HIGH-mode) and than fp8 DoubleRow (no per-column win, more throttle).
"""

import os
import sys

import numpy as np

for _p in ("/opt/trn_rl_repo", "/root/.axon_site/_ro/trn_rl_repo"):
    if os.path.isdir(_p) and _p not in sys.path:
        sys.path.insert(0, _p)

import ml_dtypes  # noqa: E402
import concourse.bass as bass  # noqa: E402
import concourse.mybir as mybir  # noqa: E402
from concourse import tile  # noqa: E402
from concourse.bass_utils import run_bass_kernel_spmd  # noqa: E402

# Problem shapes (hardcoded per contest rules).
B, S, H = 64, 512, 768
N, E = 128, 1024
GH, FH, L = 128, 256, 2
NCORES = 8
BL = B // NCORES  # graphs per core
SC = S // 128     # subtoken chunks per graph
HC = H // 128     # BERT-hidden chunks
FC = (H + GH) // 128  # concat-feature chunks for the FC head

# mega-tensor column offsets (bf16)
MEG_LHT = 0             # [HC*S] = 3072: lhT[p, hc*S + t] = lh[t, hc*128+p]
MEG_PG = HC * S         # [SC*N] = 512: P' (one-hot * invc * dinv), token-major
MEG_AH = MEG_PG + SC * N  # [N] = 128: \hat A row block
MEG_W = MEG_AH + N      # 3712 total

# consts column offsets (bf16); first CTA_W cols form the phase-A tile
C_W1 = 0                      # [HC*GH] = 768: [p, hc*128+j] = W1[hc*128+p, j]
C_WRB = C_W1 + HC * GH        # [HC*128] = 768: [p, hc*128+m] = wr[hc*128+p]
CTA_W = C_WRB + HC * 128      # 1536 = phase-A consts (W1 + wr broadcast)
C_W2 = CTA_W                  # [GH]
C_WF1 = C_W2 + GH             # [FC*2*128] = 1792
C_WF2 = C_WF1 + FC * 2 * 128  # [2*L] = 4
C_CLS = C_WF2 + 2 * L         # [HC*BL] = 48
C_MEAN = C_CLS + HC * BL      # [1]
C_IDENT = C_MEAN + 8          # [128]
C_W = C_IDENT + 128

f32 = mybir.dt.float32
bf16 = mybir.dt.bfloat16
fp8 = mybir.dt.float8e4
AFT = mybir.ActivationFunctionType
ALU = mybir.AluOpType
BF16 = ml_dtypes.bfloat16

_CACHE = {}


def _split_multi_waits(nc: bass.Bass) -> int:
    """Walrus in this container accepts one sync-wait per instruction; split
    extra waits into single-wait EventSemaphore nops just before it."""
    n_split = 0
    for fn in nc.m.functions:
        for blk in fn.blocks:
            new_instrs = []
            changed = False
            for inst in blk.instructions:
                si = getattr(inst, "sync_info", None)
                if si is not None and si.on_wait is not None and len(si.on_wait) > 1:
                    waits = list(si.on_wait)
                    for j, w in enumerate(waits[:-1]):
                        ev = mybir.InstEventSemaphore(
                            name=f"{inst.name}_ws{j}",
                            ins=[], outs=[],
                            engine=inst.engine,
                            sync_info=mybir.SyncInfo(on_wait=[w], on_update=[]),
                        )
                        new_instrs.append(ev)
                    inst.sync_info = mybir.SyncInfo(
                        on_wait=[waits[-1]], on_update=list(si.on_update))
                    n_split += 1
                    changed = True
                new_instrs.append(inst)
            if changed:
                blk.instructions = new_instrs
    return n_split


def build_program(br_val: float, b1_zero: bool, b2_zero: bool,
                  bf1_zero: bool, bf2_zero: bool) -> bass.Bass:
    nc = bass.Bass()

    meg_d = nc.declare_dram_parameter("meg", [BL, 128, MEG_W], bf16, isOutput=False)
    consts_d = nc.declare_dram_parameter("consts", [128, C_W], bf16, isOutput=False)
    b1b_d = nc.declare_dram_parameter("b1b", [128, GH], f32, isOutput=False)
    b2b_d = nc.declare_dram_parameter("b2b", [128, GH], f32, isOutput=False)
    bf1b_d = nc.declare_dram_parameter("bf1b", [128, 2], f32, isOutput=False)
    bf2b_d = nc.declare_dram_parameter("bf2b", [L, 1], f32, isOutput=False)
    out_d = nc.declare_dram_parameter("out", [L, BL], f32, isOutput=True)

    with tile.TileContext(nc) as tc:
        with (
            tc.tile_pool(name="const", bufs=1) as cpool,
            tc.tile_pool(name="megp", bufs=BL) as megpool,
            tc.tile_pool(name="work", bufs=3) as wpool,
            tc.tile_pool(name="psY", bufs=2, space="PSUM") as psY,
            tc.tile_pool(name="psL", bufs=1, space="PSUM") as psL,
            tc.tile_pool(name="psB", bufs=5, space="PSUM") as psB,
        ):
            # phase-A consts ride the ACT ring first (idle at start, and the
            # DMA engines are free before the meg stream ramps); ctB (only
            # needed by the GCN/head stages) is emitted after m1/m3.
            ctA = cpool.tile([128, CTA_W], bf16)
            nc.scalar.dma_start(ctA[:], consts_d[:, 0:CTA_W])
            ctB = cpool.tile([128, C_W - CTA_W], bf16)
            b1t = b2t = bf1t = bf2t = None
            if not b1_zero:
                b1t = cpool.tile([128, GH], f32, name="b1t")
                nc.scalar.dma_start(b1t[:], b1b_d[:])
            if not b2_zero:
                b2t = cpool.tile([128, GH], f32, name="b2t")
                nc.scalar.dma_start(b2t[:], b2b_d[:])
            if not bf1_zero:
                bf1t = cpool.tile([128, 2], f32, name="bf1t")
                nc.scalar.dma_start(bf1t[:], bf1b_d[:])
            if not bf2_zero:
                bf2t = cpool.tile([L, 1], f32, name="bf2t")
                nc.scalar.dma_start(bf2t[:], bf2b_d[:])
            catT6 = cpool.tile([128, BL], bf16)
            h1r = cpool.tile([128, 2, BL], bf16)

            # meg delivery: 8 single transfers in consumption order split
            # over the two HWDGE rings (evens on sync, odds on ACT); the
            # late odd transfers are emitted mid-loop so the XBAR
            # transposes slot into the ring FIFO ahead of them.
            megs = []
            for g in range(BL):
                m = megpool.tile([128, MEG_W], bf16, tag=f"m{g}", bufs=1,
                                 name=f"m{g}")
                if g in (1, 3):
                    nc.scalar.dma_start(m[:], meg_d[g])
                else:
                    nc.sync.dma_start(m[:], meg_d[g])
                megs.append(m)
            nc.scalar.dma_start(ctB[:], consts_d[:, CTA_W:C_W])

            W2c = ctB[:, C_W2 - CTA_W:C_W2 - CTA_W + GH]
            MEAN = ctB[:, C_MEAN - CTA_W:C_MEAN - CTA_W + 1]
            IDENT = ctB[:, C_IDENT - CTA_W:C_IDENT - CTA_W + 128]

            yT_ps = [None] * BL
            lg_ps = [None] * BL
            gate_sb = [None] * BL
            y_sb = [None] * BL
            t1sb = [None] * BL
            x1 = [None] * BL
            x1t = [None] * BL
            t2sb = [None] * BL
            x2 = [None] * BL

            def relu_to(out_sb, z_ps, bias_tile, tag):
                if bias_tile is None:
                    nc.vector.tensor_scalar_max(out_sb[:], z_ps[:], 0.0)
                else:
                    tmp = wpool.tile([128, GH], f32, tag=tag + "b", bufs=2,
                                     name=tag + "b")
                    nc.vector.tensor_tensor(tmp[:], z_ps[:], bias_tile[:],
                                            ALU.add)
                    nc.vector.tensor_scalar_max(out_sb[:], tmp[:], 0.0)

            ygsb2 = [None] * (BL // 2)
            y2 = [None] * (BL // 2)

            def gate_into_y(g):
                """multiply broadcast sigmoid gates into yT, XBAR per pair."""
                k, half = g // 2, g % 2
                if half == 0:
                    ygsb2[k] = wpool.tile([128, 2, S], bf16, tag="ygsb",
                                          bufs=2, name="ygsb")
                nc.vector.tensor_tensor(ygsb2[k][:, half, :], yT_ps[g][:],
                                        gate_sb[g][:], ALU.mult)
                if half == 1:
                    y2[k] = wpool.tile([128, 2 * SC, 128], bf16, tag="ysb",
                                       bufs=BL // 2, name="y_sb")
                    # all XBAR transposes on the ACT ring (free of bulk
                    # traffic after m1/m3), clearing in data-ready order
                    nc.scalar.dma_start(y2[k][:], ygsb2[k][:], transpose=True)
                    y_sb[2 * k] = y2[k][:, 0:SC, :]
                    y_sb[2 * k + 1] = y2[k][:, SC:2 * SC, :]

            def pool_g(g):
                t1_ps = psB.tile([128, GH], f32, tag="mm", name="t1_ps")
                for c in range(SC):
                    nc.tensor.matmul(
                        t1_ps[:],
                        megs[g][:, MEG_PG + c * N:MEG_PG + (c + 1) * N],
                        y_sb[g][:, c, :], start=(c == 0), stop=(c == SC - 1))
                t1sb[g] = wpool.tile([128, GH], bf16, tag="t1sb", bufs=BL,
                                     name="t1sb")
                nc.vector.tensor_copy(t1sb[g][:], t1_ps[:])

            def z1_g(g):
                z_ps = psB.tile([128, GH], f32, tag="mm", name="z_ps")
                nc.tensor.matmul(z_ps[:], megs[g][:, MEG_AH:MEG_AH + N],
                                 t1sb[g][:], start=True, stop=True)
                x1[g] = wpool.tile([128, GH], bf16, tag="x1", bufs=BL,
                                   name="x1")
                relu_to(x1[g], z_ps, b1t, "x1")

            def xt_g(g):
                xt_ps = psB.tile([128, GH], bf16, tag="mm", name="xt_ps")
                nc.tensor.transpose(xt_ps[:], x1[g][:], IDENT)
                x1t[g] = wpool.tile([128, GH], bf16, tag="x1t", bufs=BL,
                                    name="x1t")
                nc.vector.tensor_copy(x1t[g][:], xt_ps[:])

            def w2_g(g):
                t2_ps = psB.tile([128, GH], f32, tag="mm", name="t2_ps")
                nc.tensor.matmul(t2_ps[:], x1t[g][:], W2c,
                                 start=True, stop=True)
                t2sb[g] = wpool.tile([128, GH], bf16, tag="t2sb", bufs=BL,
                                     name="t2sb")
                nc.scalar.copy(t2sb[g][:], t2_ps[:])

            def z2_g(g):
                z2_ps = psB.tile([128, GH], f32, tag="mm", name="z2_ps")
                nc.tensor.matmul(z2_ps[:], megs[g][:, MEG_AH:MEG_AH + N],
                                 t2sb[g][:], start=True, stop=True)
                x2[g] = wpool.tile([128, GH], bf16, tag="x2", bufs=BL,
                                   name="x2")
                relu_to(x2[g], z2_ps, b2t, "x2")

            def mp_g(g):
                mp_ps = psB.tile([128, 1], f32, tag="mm", name="mp_ps")
                nc.tensor.matmul(mp_ps[:], x2[g][:], MEAN,
                                 start=True, stop=True)
                nc.vector.tensor_copy(catT6[:, g:g + 1], mp_ps[:])

            # ---- unified software pipeline: projection/gate (steps 0..8)
            # with pooling+GCN stages for finished pairs filling PE slack ----
            STAGES = [pool_g, z1_g, xt_g, w2_g, z2_g, mp_g]
            for s in range(BL + 7):
                # deep stages first (their data has been ready the longest);
                # pool (stage 0) last — its XBAR data is the freshest.
                for si in range(len(STAGES) - 1, 0, -1):
                    k2 = s - 3 - si  # == 2*k when this pair's stage is due
                    if k2 >= 0 and k2 % 2 == 0 and k2 < BL:
                        STAGES[si](k2)
                        STAGES[si](k2 + 1)
                if s < BL:
                    if s >= 1:
                        gate_sb[s - 1] = wpool.tile([128, S], bf16, tag="gt",
                                                    bufs=2, name="gate_sb")
                        nc.scalar.activation(gate_sb[s - 1][:],
                                             lg_ps[s - 1][:], AFT.Sigmoid,
                                             bias=float(br_val))
                    yT_ps[s] = psY.tile([128, S], f32, tag="yt", name="yT_ps")
                    lg_ps[s] = psL.tile([128, S], f32, tag="lg", name="lg_ps")
                    for hc in range(HC):
                        lht_c = megs[s][:, MEG_LHT + hc * S:
                                        MEG_LHT + (hc + 1) * S]
                        nc.tensor.matmul(
                            yT_ps[s][:],
                            ctA[:, C_W1 + hc * GH:C_W1 + (hc + 1) * GH],
                            lht_c, start=(hc == 0), stop=(hc == HC - 1))
                        nc.tensor.matmul(
                            lg_ps[s][:],
                            ctA[:, C_WRB + hc * 128:C_WRB + (hc + 1) * 128],
                            lht_c, start=(hc == 0), stop=(hc == HC - 1))
                    if s >= 1:
                        gate_into_y(s - 1)
                if s == BL:
                    gate_sb[BL - 1] = wpool.tile([128, S], bf16, tag="gt",
                                                 bufs=2, name="gate_sb")
                    nc.scalar.activation(gate_sb[BL - 1][:],
                                         lg_ps[BL - 1][:], AFT.Sigmoid,
                                         bias=float(br_val))
                    gate_into_y(BL - 1)
                if s >= 3 and (s - 3) % 2 == 0 and s - 3 < BL:
                    pool_g(s - 3)
                    pool_g(s - 2)

            # ---------- FC head over all BL graphs ----------
            h1_ps = []
            for hh in range(2):
                hp = psB.tile([128, BL], f32, tag="mm", name=f"h1_ps{hh}")
                for c in range(FC):
                    lhsT = ctB[:, C_WF1 - CTA_W + (c * 2 + hh) * 128:
                               C_WF1 - CTA_W + (c * 2 + hh + 1) * 128]
                    rhs = (ctB[:, C_CLS - CTA_W + c * BL:C_CLS - CTA_W + (c + 1) * BL]
                           if c < HC else catT6[:])
                    nc.tensor.matmul(hp[:], lhsT, rhs, start=(c == 0),
                                     stop=(c == FC - 1))
                h1_ps.append(hp)
            for hh in range(2):
                if bf1t is None:
                    nc.vector.tensor_scalar_max(h1r[:, hh, :], h1_ps[hh][:],
                                                0.0)
                else:
                    nc.vector.tensor_scalar(h1r[:, hh, :], h1_ps[hh][:],
                                            bf1t[:, hh:hh + 1], 0.0,
                                            ALU.add, ALU.max)
            out_ps = psB.tile([L, BL], f32, tag="mm", name="out_ps")
            for hh in range(2):
                nc.tensor.matmul(out_ps[:],
                                 ctB[:, C_WF2 - CTA_W + hh * L:C_WF2 - CTA_W + (hh + 1) * L],
                                 h1r[:, hh, :], start=(hh == 0),
                                 stop=(hh == 1))
            outs = cpool.tile([L, BL], f32)
            if bf2t is None:
                nc.vector.tensor_copy(outs[:], out_ps[:])
            else:
                nc.vector.tensor_scalar_add(outs[:], out_ps[:], bf2t[:])
            nc.sync.dma_start(out_d[:], outs[:])

    _split_multi_waits(nc)
    return nc


def _prepare_in_maps(inputs):
    lh = np.ascontiguousarray(np.asarray(inputs["last_hidden"], dtype=np.float32))
    submap = np.asarray(inputs["submap"]).astype(np.int64)
    edge_index = np.asarray(inputs["edge_index"]).astype(np.int64)
    assert lh.shape == (B, S, H)
    assert int(inputs.get("num_nodes", N)) == N

    wr = np.asarray(inputs["wr"], dtype=np.float32)
    br = float(np.asarray(inputs["br"], dtype=np.float32))
    W1 = np.asarray(inputs["W1"], dtype=np.float32)
    b1 = np.asarray(inputs["b1"], dtype=np.float32)
    W2 = np.asarray(inputs["W2"], dtype=np.float32)
    b2 = np.asarray(inputs["b2"], dtype=np.float32)
    Wf1 = np.asarray(inputs["Wf1"], dtype=np.float32)
    bf1 = np.asarray(inputs["bf1"], dtype=np.float32)
    Wf2 = np.asarray(inputs["Wf2"], dtype=np.float32)
    bf2 = np.asarray(inputs["bf2"], dtype=np.float32)

    # ---- host-side index prep: adjacency, degrees, counts ----
    src = edge_index[:, 0, :]
    dst = edge_index[:, 1, :]
    flat = (np.arange(B, dtype=np.int64)[:, None] * (N * N) + src * N + dst)
    A = np.bincount(flat.reshape(-1), minlength=B * N * N).astype(np.float32)
    A = A.reshape(B, N, N) + np.eye(N, dtype=np.float32)[None]
    deg = A.sum(axis=1)                      # in-degree incl self-loops
    dinv = 1.0 / np.sqrt(deg)
    ahat = A * dinv[:, :, None] * dinv[:, None, :]

    cflat = np.arange(B, dtype=np.int64)[:, None] * N + submap
    cnt = np.bincount(cflat.reshape(-1), minlength=B * N).astype(np.float32)
    invc = 1.0 / np.maximum(cnt.reshape(B, N), 1.0)

    P = (submap[:, :, None] == np.arange(N)[None, None, :]).astype(np.float32)
    P *= (invc * dinv)[:, None, :]

    # ---- mega-tensor assembly (bf16) ----
    lht = lh.astype(BF16).reshape(B, S, HC, 128).transpose(0, 3, 2, 1)
    p_r = P.astype(BF16).reshape(B, SC, 128, N).transpose(0, 2, 1, 3)
    meg = np.empty((B, 128, MEG_W), dtype=BF16)
    meg[:, :, MEG_LHT:MEG_PG] = lht.reshape(B, 128, HC * S)
    meg[:, :, MEG_PG:MEG_AH] = p_r.reshape(B, 128, SC * N)
    meg[:, :, MEG_AH:MEG_W] = ahat.astype(BF16)

    # ---- consts (bf16), cls block differs per core ----
    consts = np.zeros((128, C_W), dtype=np.float32)
    consts[:, C_W1:C_W1 + HC * GH] = (
        W1.reshape(HC, 128, GH).transpose(1, 0, 2).reshape(128, HC * GH))
    consts[:, C_W2:C_W2 + GH] = W2
    consts[:, C_WF1:C_WF1 + FC * 2 * 128] = (
        Wf1.reshape(FC, 128, 2, 128).transpose(1, 0, 2, 3).reshape(128, -1))
    consts[:, C_WF2:C_WF2 + 2 * L] = (
        Wf2.reshape(2, 128, L).transpose(1, 0, 2).reshape(128, 2 * L))
    consts[:, C_MEAN] = 1.0 / N
    consts[:, C_WRB:C_WRB + HC * 128] = np.repeat(
        wr.reshape(HC, 128).T, 128, axis=1).reshape(128, HC * 128)
    consts[:, C_IDENT:C_IDENT + 128] = np.eye(128, dtype=np.float32)

    b1b = np.ascontiguousarray(np.broadcast_to(b1, (128, GH)).astype(np.float32))
    b2b = np.ascontiguousarray(np.broadcast_to(b2, (128, GH)).astype(np.float32))
    bf1b = np.ascontiguousarray(bf1.reshape(2, 128).T.astype(np.float32))
    bf2b = np.ascontiguousarray(bf2.reshape(L, 1).astype(np.float32))

    in_maps = []
    for i in range(NCORES):
        sl = slice(i * BL, (i + 1) * BL)
        ci = consts.copy()
        ci[:, C_CLS:C_CLS + HC * BL] = (
            lh[sl, 0, :].reshape(BL, HC, 128).transpose(2, 1, 0)
            .reshape(128, HC * BL))
        cb = ci.astype(BF16)
        in_maps.append({
            "meg": np.ascontiguousarray(meg[sl]),
            "consts": cb,
            "b1b": b1b, "b2b": b2b, "bf1b": bf1b, "bf2b": bf2b,
        })
    flags = (br, bool(np.all(b1 == 0)), bool(np.all(b2 == 0)),
             bool(np.all(bf1 == 0)), bool(np.all(bf2 == 0)))
    return in_maps, flags


def _run(inputs, trace=False):
    in_maps, flags = _prepare_in_maps(inputs)
    key = ("prog",) + flags
    if key not in _CACHE:
        _CACHE[key] = build_program(*flags)
    nc = _CACHE[key]
    res = run_bass_kernel_spmd(nc, in_maps, list(range(NCORES)), trace=trace)
    out = np.concatenate(
        [np.asarray(res.results[i]["out"]).T for i in range(NCORES)],
        axis=0).astype(np.float32)
    return out, res


def kernel(**inputs) -> np.ndarray:
    out, _ = _run(inputs, trace=False)
    return out



# revision 2
# speedup vs baseline: 1.2251x; 1.2251x over previous
"""Trainium2 Bass kernel for BioBERT-ARG-GNN (gated pooling + 2-layer GCN + MLP head).

Strategy: pure data parallel over batch B=64 across 8 NeuronCores (8
graphs per core).  Host precomputes index-derived structures (one-hot
pooling matrix P' with 1/cnt, D^-1/2 and the fp8 weight scale folded in,
normalized adjacency \hat A = D^-1/2 (A+I) D^-1/2) and ships them bf16,
together with an fp8(e4m3) TRANSPOSED copy of last_hidden
(lhT, [hidden, tokens]).  The transposed layout lets BOTH the gate
logits and the W1 projection run on the PE with the contraction over
the hidden dim, in fp8 DoubleRow mode (256-deep contraction per pass):

    yT[gh, t] = sum_j W1c^T  @ lhT_j    (3 DR matmuls, free=512)
    lg[:, t]  = sum_j wrbc^T @ lhT_j    (3 DR matmuls; stationary =
                                         wr chunk broadcast to 128
                                         columns, so lg is born
                                         partition-broadcast)

W1 and wr are scaled by SC_W=64 on the host so fp8 stays in the normal
range; the sigmoid un-scales via the ACT scale operand and the yT path
un-scales through P'.  sigmoid(lg) on ACT -> [128, 512] gates; DVE
multiplies them into yT while casting to bf16; ONE DMA-XBAR transpose
per graph PAIR turns ygT into token-major y chunks; pooling then
contracts tokens directly: t1 = P'^T (g*y) = pool(gated lh) @ W1 — no
nf materialization and no PE transposes.  GCN layers use \hat A as the
stationary operand; the FC head is batched over all 8 graphs with zero
transposes (cls ships pre-transposed; the [2, BL] output is transposed
back on the host).

Scheduling: a single software pipeline where step s issues graph s's
projection matmuls (paced by that graph's DMA) while pooling + the
five GCN stages for earlier graph pairs fill the PE slack.  DMA rings:
sync HWDGE carries meg8/megb 0,2,4,5,6,7; ACT HWDGE carries the fp8
consts, meg8/megb 1,3, the bf16 consts, and all XBAR transposes (kept
free of late bulk traffic so transposes clear in data-ready order; the
sequencers park blocked instructions, so ring order must match
data-ready order).
"""

import os
import sys

import numpy as np

for _p in ("/opt/trn_rl_repo", "/root/.axon_site/_ro/trn_rl_repo"):
    if os.path.isdir(_p) and _p not in sys.path:
        sys.path.insert(0, _p)

import ml_dtypes  # noqa: E402
import concourse.bass as bass  # noqa: E402
import concourse.mybir as mybir  # noqa: E402
from concourse import tile  # noqa: E402
from concourse.bass_utils import run_bass_kernel_spmd  # noqa: E402

# Problem shapes (hardcoded per contest rules).
B, S, H = 64, 512, 768
N, E = 128, 1024
GH, FH, L = 128, 256, 2
NCORES = 8
BL = B // NCORES  # graphs per core
SC = S // 128     # subtoken chunks per graph
HC = H // 128     # BERT-hidden chunks
FC = (H + GH) // 128  # concat-feature chunks for the FC head
SC_W = 64.0       # fp8 weight scale (W1, wr); folded back via P'/sigmoid

# fp8 consts column offsets
C8_W1 = 0                 # [HC*GH] = 768: [p, hc, j] = SC_W*W1[hc*128+p, j]
C8_WRB = HC * GH          # [HC*128]: [p, hc, m] = SC_W*wr[hc*128+p]
C8_W = C8_WRB + HC * 128  # 1536

# bf16 consts column offsets
C_W2 = 0                      # [GH]
C_WF1 = C_W2 + GH             # [FC*2*128] = 1792
C_WF2 = C_WF1 + FC * 2 * 128  # [2*L] = 4
C_CLS = C_WF2 + 2 * L         # [HC*BL] = 48
C_MEAN = C_CLS + HC * BL      # [1]
C_IDENT = C_MEAN + 8          # [128]
C_W = C_IDENT + 128

f32 = mybir.dt.float32
bf16 = mybir.dt.bfloat16
fp8 = mybir.dt.float8e4
AFT = mybir.ActivationFunctionType
ALU = mybir.AluOpType
MPM = mybir.MatmulPerfMode
BF16 = ml_dtypes.bfloat16
E4M3 = ml_dtypes.float8_e4m3

_CACHE = {}


def _split_multi_waits(nc: bass.Bass) -> int:
    """Walrus in this container accepts one sync-wait per instruction; split
    extra waits into single-wait EventSemaphore nops just before it."""
    n_split = 0
    for fn in nc.m.functions:
        for blk in fn.blocks:
            new_instrs = []
            changed = False
            for inst in blk.instructions:
                si = getattr(inst, "sync_info", None)
                if si is not None and si.on_wait is not None and len(si.on_wait) > 1:
                    waits = list(si.on_wait)
                    for j, w in enumerate(waits[:-1]):
                        ev = mybir.InstEventSemaphore(
                            name=f"{inst.name}_ws{j}",
                            ins=[], outs=[],
                            engine=inst.engine,
                            sync_info=mybir.SyncInfo(on_wait=[w], on_update=[]),
                        )
                        new_instrs.append(ev)
                    inst.sync_info = mybir.SyncInfo(
                        on_wait=[waits[-1]], on_update=list(si.on_update))
                    n_split += 1
                    changed = True
                new_instrs.append(inst)
            if changed:
                blk.instructions = new_instrs
    return n_split


def build_program(br_val: float, b1_zero: bool, b2_zero: bool,
                  bf1_zero: bool, bf2_zero: bool) -> bass.Bass:
    nc = bass.Bass()

    meg8_d = nc.declare_dram_parameter("meg8", [BL, 128, HC, S], fp8,
                                       isOutput=False)
    megb_d = nc.declare_dram_parameter("megb", [BL, 128, (SC + 1) * N], bf16,
                                       isOutput=False)
    ct8_d = nc.declare_dram_parameter("ct8", [128, C8_W], fp8, isOutput=False)
    consts_d = nc.declare_dram_parameter("consts", [128, C_W], bf16,
                                         isOutput=False)
    b1b_d = nc.declare_dram_parameter("b1b", [128, GH], f32, isOutput=False)
    b2b_d = nc.declare_dram_parameter("b2b", [128, GH], f32, isOutput=False)
    bf1b_d = nc.declare_dram_parameter("bf1b", [128, 2], f32, isOutput=False)
    bf2b_d = nc.declare_dram_parameter("bf2b", [L, 1], f32, isOutput=False)
    out_d = nc.declare_dram_parameter("out", [L, BL], f32, isOutput=True)

    with tile.TileContext(nc) as tc:
        with (
            tc.tile_pool(name="const", bufs=1) as cpool,
            tc.tile_pool(name="megp", bufs=BL) as megpool,
            tc.tile_pool(name="work", bufs=3) as wpool,
            tc.tile_pool(name="psY", bufs=2, space="PSUM") as psY,
            tc.tile_pool(name="psL", bufs=1, space="PSUM") as psL,
            tc.tile_pool(name="psB", bufs=5, space="PSUM") as psB,
        ):
            # fp8 consts ride the ACT ring first (idle at start, and the
            # DMA engines are free before the meg stream ramps); ctB (only
            # needed by the GCN/head stages) is emitted after m1/m3.
            ct8 = cpool.tile([128, HC, GH + 128], fp8)
            nc.scalar.dma_start(ct8[:], ct8_d[:])
            ctB = cpool.tile([128, C_W], bf16)
            b1t = b2t = bf1t = bf2t = None
            if not b1_zero:
                b1t = cpool.tile([128, GH], f32, name="b1t")
                nc.scalar.dma_start(b1t[:], b1b_d[:])
            if not b2_zero:
                b2t = cpool.tile([128, GH], f32, name="b2t")
                nc.scalar.dma_start(b2t[:], b2b_d[:])
            if not bf1_zero:
                bf1t = cpool.tile([128, 2], f32, name="bf1t")
                nc.scalar.dma_start(bf1t[:], bf1b_d[:])
            if not bf2_zero:
                bf2t = cpool.tile([L, 1], f32, name="bf2t")
                nc.scalar.dma_start(bf2t[:], bf2b_d[:])
            catT6 = cpool.tile([128, BL], bf16)
            h1r = cpool.tile([128, 2, BL], bf16)

            # meg delivery: single transfers per graph in consumption order
            # split over the two HWDGE rings (evens on sync, odds on ACT);
            # the late odd transfers are emitted before ctB so the XBAR
            # transposes slot into the ACT ring FIFO behind all bulk.
            megs8 = []
            megsb = []
            for g in range(BL):
                m8 = megpool.tile([128, HC, S], fp8, tag=f"m8{g}", bufs=1,
                                  name=f"m8{g}")
                mb = megpool.tile([128, SC + 1, N], bf16, tag=f"mb{g}",
                                  bufs=1, name=f"mb{g}")
                if g in (1, 3):
                    nc.scalar.dma_start(m8[:], meg8_d[g])
                    nc.scalar.dma_start(mb[:], megb_d[g])
                else:
                    nc.sync.dma_start(m8[:], meg8_d[g])
                    nc.sync.dma_start(mb[:], megb_d[g])
                megs8.append(m8)
                megsb.append(mb)
            nc.scalar.dma_start(ctB[:], consts_d[:])

            W1c = ct8[:, :, 0:GH]          # [128, HC, GH]
            WRBc = ct8[:, :, GH:GH + 128]  # [128, HC, 128]
            W2c = ctB[:, C_W2:C_W2 + GH]
            MEAN = ctB[:, C_MEAN:C_MEAN + 1]
            IDENT = ctB[:, C_IDENT:C_IDENT + 128]

            yT_ps = [None] * BL
            lg_ps = [None] * BL
            gate_sb = [None] * BL
            y_sb = [None] * BL
            t1sb = [None] * BL
            x1 = [None] * BL
            x1t = [None] * BL
            t2sb = [None] * BL
            x2 = [None] * BL

            def relu_to(out_sb, z_ps, bias_tile, tag):
                if bias_tile is None:
                    nc.vector.tensor_scalar_max(out_sb[:], z_ps[:], 0.0)
                else:
                    tmp = wpool.tile([128, GH], f32, tag=tag + "b", bufs=2,
                                     name=tag + "b")
                    nc.vector.tensor_tensor(tmp[:], z_ps[:], bias_tile[:],
                                            ALU.add)
                    nc.vector.tensor_scalar_max(out_sb[:], tmp[:], 0.0)

            ygsb2 = [None] * (BL // 2)
            y2 = [None] * (BL // 2)

            def gate_into_y(g):
                """multiply broadcast sigmoid gates into yT, XBAR per pair."""
                k, half = g // 2, g % 2
                if half == 0:
                    ygsb2[k] = wpool.tile([128, 2, S], bf16, tag="ygsb",
                                          bufs=2, name="ygsb")
                nc.vector.tensor_tensor(ygsb2[k][:, half, :], yT_ps[g][:],
                                        gate_sb[g][:], ALU.mult)
                if half == 1:
                    y2[k] = wpool.tile([128, 2 * SC, 128], bf16, tag="ysb",
                                       bufs=BL // 2, name="y_sb")
                    # all XBAR transposes on the ACT ring (free of bulk
                    # traffic after the consts), clearing in data-ready order
                    nc.scalar.dma_start(y2[k][:], ygsb2[k][:], transpose=True)
                    y_sb[2 * k] = y2[k][:, 0:SC, :]
                    y_sb[2 * k + 1] = y2[k][:, SC:2 * SC, :]

            def pool_g(g):
                t1_ps = psB.tile([128, GH], f32, tag="mm", name="t1_ps")
                for c in range(SC):
                    nc.tensor.matmul(
                        t1_ps[:],
                        megsb[g][:, c, :],
                        y_sb[g][:, c, :], start=(c == 0), stop=(c == SC - 1))
                t1sb[g] = wpool.tile([128, GH], bf16, tag="t1sb", bufs=BL,
                                     name="t1sb")
                nc.vector.tensor_copy(t1sb[g][:], t1_ps[:])

            def z1_g(g):
                z_ps = psB.tile([128, GH], f32, tag="mm", name="z_ps")
                nc.tensor.matmul(z_ps[:], megsb[g][:, SC, :],
                                 t1sb[g][:], start=True, stop=True)
                x1[g] = wpool.tile([128, GH], bf16, tag="x1", bufs=BL,
                                   name="x1")
                relu_to(x1[g], z_ps, b1t, "x1")

            def xt_g(g):
                xt_ps = psB.tile([128, GH], bf16, tag="mm", name="xt_ps")
                nc.tensor.transpose(xt_ps[:], x1[g][:], IDENT)
                x1t[g] = wpool.tile([128, GH], bf16, tag="x1t", bufs=BL,
                                    name="x1t")
                nc.vector.tensor_copy(x1t[g][:], xt_ps[:])

            def w2_g(g):
                t2_ps = psB.tile([128, GH], f32, tag="mm", name="t2_ps")
                nc.tensor.matmul(t2_ps[:], x1t[g][:], W2c,
                                 start=True, stop=True)
                t2sb[g] = wpool.tile([128, GH], bf16, tag="t2sb", bufs=BL,
                                     name="t2sb")
                nc.scalar.copy(t2sb[g][:], t2_ps[:])

            def z2_g(g):
                z2_ps = psB.tile([128, GH], f32, tag="mm", name="z2_ps")
                nc.tensor.matmul(z2_ps[:], megsb[g][:, SC, :],
                                 t2sb[g][:], start=True, stop=True)
                x2[g] = wpool.tile([128, GH], bf16, tag="x2", bufs=BL,
                                   name="x2")
                relu_to(x2[g], z2_ps, b2t, "x2")

            def mp_g(g):
                mp_ps = psB.tile([128, 1], f32, tag="mm", name="mp_ps")
                nc.tensor.matmul(mp_ps[:], x2[g][:], MEAN,
                                 start=True, stop=True)
                nc.vector.tensor_copy(catT6[:, g:g + 1], mp_ps[:])

            # ---- unified software pipeline: projection/gate (steps 0..8)
            # with pooling+GCN stages for finished pairs filling PE slack ----
            STAGES = [pool_g, z1_g, xt_g, w2_g, z2_g, mp_g]
            for s in range(BL + 7):
                # deep stages first (their data has been ready the longest);
                # pool (stage 0) last — its XBAR data is the freshest.
                for si in range(len(STAGES) - 1, 0, -1):
                    k2 = s - 3 - si  # == 2*k when this pair's stage is due
                    if k2 >= 0 and k2 % 2 == 0 and k2 < BL:
                        STAGES[si](k2)
                        STAGES[si](k2 + 1)
                if s < BL:
                    if s >= 1:
                        gate_sb[s - 1] = wpool.tile([128, S], bf16, tag="gt",
                                                    bufs=2, name="gate_sb")
                        nc.scalar.activation(gate_sb[s - 1][:],
                                             lg_ps[s - 1][:], AFT.Sigmoid,
                                             bias=float(br_val),
                                             scale=1.0 / SC_W)
                    yT_ps[s] = psY.tile([128, S], f32, tag="yt", name="yT_ps")
                    lg_ps[s] = psL.tile([128, S], f32, tag="lg", name="lg_ps")
                    for j in range(HC // 2):
                        lht_j = megs8[s][:, 2 * j:2 * j + 2, :]
                        nc.tensor.matmul(
                            yT_ps[s][:], W1c[:, 2 * j:2 * j + 2, :],
                            lht_j, start=(j == 0), stop=(j == HC // 2 - 1),
                            perf_mode=MPM.DoubleRow)
                        nc.tensor.matmul(
                            lg_ps[s][:], WRBc[:, 2 * j:2 * j + 2, :],
                            lht_j, start=(j == 0), stop=(j == HC // 2 - 1),
                            perf_mode=MPM.DoubleRow)
                    if s >= 1:
                        gate_into_y(s - 1)
                if s == BL:
                    gate_sb[BL - 1] = wpool.tile([128, S], bf16, tag="gt",
                                                 bufs=2, name="gate_sb")
                    nc.scalar.activation(gate_sb[BL - 1][:],
                                         lg_ps[BL - 1][:], AFT.Sigmoid,
                                         bias=float(br_val),
                                         scale=1.0 / SC_W)
                    gate_into_y(BL - 1)
                if s >= 3 and (s - 3) % 2 == 0 and s - 3 < BL:
                    pool_g(s - 3)
                    pool_g(s - 2)

            # ---------- FC head over all BL graphs ----------
            h1_ps = []
            for hh in range(2):
                hp = psB.tile([128, BL], f32, tag="mm", name=f"h1_ps{hh}")
                for c in range(FC):
                    lhsT = ctB[:, C_WF1 + (c * 2 + hh) * 128:
                               C_WF1 + (c * 2 + hh + 1) * 128]
                    rhs = (ctB[:, C_CLS + c * BL:C_CLS + (c + 1) * BL]
                           if c < HC else catT6[:])
                    nc.tensor.matmul(hp[:], lhsT, rhs, start=(c == 0),
                                     stop=(c == FC - 1))
                h1_ps.append(hp)
            for hh in range(2):
                if bf1t is None:
                    nc.vector.tensor_scalar_max(h1r[:, hh, :], h1_ps[hh][:],
                                                0.0)
                else:
                    nc.vector.tensor_scalar(h1r[:, hh, :], h1_ps[hh][:],
                                            bf1t[:, hh:hh + 1], 0.0,
                                            ALU.add, ALU.max)
            out_ps = psB.tile([L, BL], f32, tag="mm", name="out_ps")
            for hh in range(2):
                nc.tensor.matmul(out_ps[:],
                                 ctB[:, C_WF2 + hh * L:C_WF2 + (hh + 1) * L],
                                 h1r[:, hh, :], start=(hh == 0),
                                 stop=(hh == 1))
            outs = cpool.tile([L, BL], f32)
            if bf2t is None:
                nc.vector.tensor_copy(outs[:], out_ps[:])
            else:
                nc.vector.tensor_scalar_add(outs[:], out_ps[:], bf2t[:])
            nc.sync.dma_start(out_d[:], outs[:])

    _split_multi_waits(nc)
    return nc


def _prepare_in_maps(inputs):
    lh = np.ascontiguousarray(np.asarray(inputs["last_hidden"], dtype=np.float32))
    submap = np.asarray(inputs["submap"]).astype(np.int64)
    edge_index = np.asarray(inputs["edge_index"]).astype(np.int64)
    assert lh.shape == (B, S, H)
    assert int(inputs.get("num_nodes", N)) == N

    wr = np.asarray(inputs["wr"], dtype=np.float32)
    br = float(np.asarray(inputs["br"], dtype=np.float32))
    W1 = np.asarray(inputs["W1"], dtype=np.float32)
    b1 = np.asarray(inputs["b1"], dtype=np.float32)
    W2 = np.asarray(inputs["W2"], dtype=np.float32)
    b2 = np.asarray(inputs["b2"], dtype=np.float32)
    Wf1 = np.asarray(inputs["Wf1"], dtype=np.float32)
    bf1 = np.asarray(inputs["bf1"], dtype=np.float32)
    Wf2 = np.asarray(inputs["Wf2"], dtype=np.float32)
    bf2 = np.asarray(inputs["bf2"], dtype=np.float32)

    # ---- host-side index prep: adjacency, degrees, counts ----
    src = edge_index[:, 0, :]
    dst = edge_index[:, 1, :]
    flat = (np.arange(B, dtype=np.int64)[:, None] * (N * N) + src * N + dst)
    A = np.bincount(flat.reshape(-1), minlength=B * N * N).astype(np.float32)
    A = A.reshape(B, N, N) + np.eye(N, dtype=np.float32)[None]
    deg = A.sum(axis=1)                      # in-degree incl self-loops
    dinv = 1.0 / np.sqrt(deg)
    ahat = A * dinv[:, :, None] * dinv[:, None, :]

    cflat = np.arange(B, dtype=np.int64)[:, None] * N + submap
    cnt = np.bincount(cflat.reshape(-1), minlength=B * N).astype(np.float32)
    invc = 1.0 / np.maximum(cnt.reshape(B, N), 1.0)

    P = (submap[:, :, None] == np.arange(N)[None, None, :]).astype(np.float32)
    P *= (invc * dinv)[:, None, :] * (1.0 / SC_W)

    # ---- mega-tensors: fp8 lhT + bf16 pool/adjacency ----
    lht = lh.astype(E4M3).reshape(B, S, HC, 128).transpose(0, 3, 2, 1)
    meg8 = np.ascontiguousarray(lht)          # [B, 128, HC, S]
    p_r = P.astype(BF16).reshape(B, SC, 128, N).transpose(0, 2, 1, 3)
    megb = np.empty((B, 128, (SC + 1) * N), dtype=BF16)
    megb[:, :, 0:SC * N] = p_r.reshape(B, 128, SC * N)
    megb[:, :, SC * N:(SC + 1) * N] = ahat.astype(BF16)

    # ---- fp8 consts: scaled W1 + broadcast wr ----
    ct8 = np.zeros((128, C8_W), dtype=np.float32)
    ct8[:, C8_W1:C8_W1 + HC * GH] = (
        (SC_W * W1).reshape(HC, 128, GH).transpose(1, 0, 2).reshape(128, -1))
    ct8[:, C8_WRB:C8_WRB + HC * 128] = np.repeat(
        (SC_W * wr).reshape(HC, 128).T, 128, axis=1).reshape(128, HC * 128)
    # interleave into [p, hc, GH+128] layout matching the SBUF tile
    ct8_t = np.empty((128, HC, GH + 128), dtype=E4M3)
    ct8_t[:, :, 0:GH] = ct8[:, 0:HC * GH].reshape(128, HC, GH).astype(E4M3)
    ct8_t[:, :, GH:] = ct8[:, HC * GH:].reshape(128, HC, 128).astype(E4M3)
    ct8_flat = np.ascontiguousarray(ct8_t.reshape(128, -1))

    # ---- bf16 consts, cls block differs per core ----
    consts = np.zeros((128, C_W), dtype=np.float32)
    consts[:, C_W2:C_W2 + GH] = W2
    consts[:, C_WF1:C_WF1 + FC * 2 * 128] = (
        Wf1.reshape(FC, 128, 2, 128).transpose(1, 0, 2, 3).reshape(128, -1))
    consts[:, C_WF2:C_WF2 + 2 * L] = (
        Wf2.reshape(2, 128, L).transpose(1, 0, 2).reshape(128, 2 * L))
    consts[:, C_MEAN] = 1.0 / N
    consts[:, C_IDENT:C_IDENT + 128] = np.eye(128, dtype=np.float32)

    b1b = np.ascontiguousarray(np.broadcast_to(b1, (128, GH)).astype(np.float32))
    b2b = np.ascontiguousarray(np.broadcast_to(b2, (128, GH)).astype(np.float32))
    bf1b = np.ascontiguousarray(bf1.reshape(2, 128).T.astype(np.float32))
    bf2b = np.ascontiguousarray(bf2.reshape(L, 1).astype(np.float32))

    in_maps = []
    for i in range(NCORES):
        sl = slice(i * BL, (i + 1) * BL)
        ci = consts.copy()
        ci[:, C_CLS:C_CLS + HC * BL] = (
            lh[sl, 0, :].reshape(BL, HC, 128).transpose(2, 1, 0)
            .reshape(128, HC * BL))
        cb = ci.astype(BF16)
        in_maps.append({
            "meg8": np.ascontiguousarray(meg8[sl]),
            "megb": np.ascontiguousarray(megb[sl]),
            "ct8": ct8_flat,
            "consts": cb,
            "b1b": b1b, "b2b": b2b, "bf1b": bf1b, "bf2b": bf2b,
        })
    flags = (br, bool(np.all(b1 == 0)), bool(np.all(b2 == 0)),
             bool(np.all(bf1 == 0)), bool(np.all(bf2 == 0)))
    return in_maps, flags


def _run(inputs, trace=False):
    in_maps, flags = _prepare_in_maps(inputs)
    key = ("prog",) + flags
    if key not in _CACHE:
        _CACHE[key] = build_program(*flags)
    nc = _CACHE[key]
    res = run_bass_kernel_spmd(nc, in_maps, list(range(NCORES)), trace=trace)
    out = np.concatenate(
        [np.asarray(res.results[i]["out"]).T for i in range(NCORES)],
        axis=0).astype(np.float32)
    return out, res


def kernel(**inputs) -> np.ndarray:
    out, _ = _run(inputs, trace=False)
    return out


# revision 16
# speedup vs baseline: 1.2358x; 1.0087x over previous
"""Trainium2 Bass kernel for BioBERT-ARG-GNN (gated pooling + 2-layer GCN + MLP head).

Strategy: pure data parallel over batch B=64 across 8 NeuronCores (8
graphs per core).  Host precomputes index-derived structures (one-hot
pooling matrix P' with 1/cnt, D^-1/2 and the fp8 weight scale folded in,
normalized adjacency \hat A = D^-1/2 (A+I) D^-1/2) and ships them bf16,
together with an fp8(e4m3) TRANSPOSED copy of last_hidden
(lhT, [hidden, tokens]).  The transposed layout lets BOTH the gate
logits and the W1 projection run on the PE with the contraction over
the hidden dim, in fp8 DoubleRow mode (256-deep contraction per pass):

    yT[gh, t] = sum_j W1c^T  @ lhT_j    (3 DR matmuls, free=512)
    lg[:, t]  = sum_j wrbc^T @ lhT_j    (3 DR matmuls; stationary =
                                         wr chunk broadcast to 128
                                         columns, so lg is born
                                         partition-broadcast)

W1 and wr are scaled by SC_W=64 on the host so fp8 stays in the normal
range; the sigmoid un-scales via the ACT scale operand and the yT path
un-scales through P'.  sigmoid(lg) on ACT -> [128, 512] gates; DVE
multiplies them into yT while casting to bf16; ONE DMA-XBAR transpose
per graph PAIR turns ygT into token-major y chunks; pooling then
contracts tokens directly: t1 = P'^T (g*y) = pool(gated lh) @ W1 — no
nf materialization and no PE transposes.  GCN layers use \hat A as the
stationary operand; the FC head is batched over all 8 graphs with zero
transposes (cls ships pre-transposed; the [2, BL] output is transposed
back on the host).

Scheduling: a single software pipeline where step s issues graph s's
projection matmuls (paced by that graph's DMA) while pooling + the
five GCN stages for earlier graph pairs fill the PE slack.  DMA rings:
sync HWDGE carries meg8/megb 0,2,4,5,6,7; ACT HWDGE carries the fp8
consts, meg8/megb 1,3, the bf16 consts, and all XBAR transposes (kept
free of late bulk traffic so transposes clear in data-ready order; the
sequencers park blocked instructions, so ring order must match
data-ready order).
"""

import os
import sys

import numpy as np

for _p in ("/opt/trn_rl_repo", "/root/.axon_site/_ro/trn_rl_repo"):
    if os.path.isdir(_p) and _p not in sys.path:
        sys.path.insert(0, _p)

import ml_dtypes  # noqa: E402
import concourse.bass as bass  # noqa: E402
import concourse.mybir as mybir  # noqa: E402
from concourse import tile  # noqa: E402
from concourse.bass_utils import run_bass_kernel_spmd  # noqa: E402

# Problem shapes (hardcoded per contest rules).
B, S, H = 64, 512, 768
N, E = 128, 1024
GH, FH, L = 128, 256, 2
NCORES = 8
BL = B // NCORES  # graphs per core
SC = S // 128     # subtoken chunks per graph
HC = H // 128     # BERT-hidden chunks
FC = (H + GH) // 128  # concat-feature chunks for the FC head
SC_W = 64.0       # fp8 weight scale (W1, wr); folded back via P'/sigmoid

# fp8 consts column offsets
C8_W1 = 0                 # [HC*GH] = 768: [p, hc, j] = SC_W*W1[hc*128+p, j]
C8_WRB = HC * GH          # [HC*128]: [p, hc, m] = SC_W*wr[hc*128+p]
C8_W = C8_WRB + HC * 128  # 1536

# merged per-graph mega tensor (fp8 cols; tail bytes hold bf16 P'+Ahat)
MEGW8 = HC * S + 2 * (SC + 1) * N  # 3072 + 1280 = 4352

# bf16 consts column offsets
C_W2 = 0                      # [GH]
C_WF1 = C_W2 + GH             # [FC*2*128] = 1792
C_WF2 = C_WF1 + FC * 2 * 128  # [2*L] = 4
C_CLS = C_WF2 + 2 * L         # [HC*BL] = 48
C_MEAN = C_CLS + HC * BL      # [1]
C_IDENT = C_MEAN + 8          # [128]
C_W = C_IDENT + 128

f32 = mybir.dt.float32
bf16 = mybir.dt.bfloat16
fp8 = mybir.dt.float8e4
AFT = mybir.ActivationFunctionType
ALU = mybir.AluOpType
MPM = mybir.MatmulPerfMode
BF16 = ml_dtypes.bfloat16
E4M3 = ml_dtypes.float8_e4m3

_CACHE = {}


def _split_multi_waits(nc: bass.Bass) -> int:
    """Walrus in this container accepts one sync-wait per instruction; split
    extra waits into single-wait EventSemaphore nops just before it."""
    n_split = 0
    for fn in nc.m.functions:
        for blk in fn.blocks:
            new_instrs = []
            changed = False
            for inst in blk.instructions:
                si = getattr(inst, "sync_info", None)
                if si is not None and si.on_wait is not None and len(si.on_wait) > 1:
                    waits = list(si.on_wait)
                    for j, w in enumerate(waits[:-1]):
                        ev = mybir.InstEventSemaphore(
                            name=f"{inst.name}_ws{j}",
                            ins=[], outs=[],
                            engine=inst.engine,
                            sync_info=mybir.SyncInfo(on_wait=[w], on_update=[]),
                        )
                        new_instrs.append(ev)
                    inst.sync_info = mybir.SyncInfo(
                        on_wait=[waits[-1]], on_update=list(si.on_update))
                    n_split += 1
                    changed = True
                new_instrs.append(inst)
            if changed:
                blk.instructions = new_instrs
    return n_split


def build_program(br_val: float, b1_zero: bool, b2_zero: bool,
                  bf1_zero: bool, bf2_zero: bool) -> bass.Bass:
    nc = bass.Bass()

    meg8_d = nc.declare_dram_parameter("meg8", [BL, 128, MEGW8], fp8,
                                       isOutput=False)
    ct8_d = nc.declare_dram_parameter("ct8", [128, C8_W], fp8, isOutput=False)
    consts_d = nc.declare_dram_parameter("consts", [128, C_W], bf16,
                                         isOutput=False)
    b1b_d = nc.declare_dram_parameter("b1b", [128, GH], f32, isOutput=False)
    b2b_d = nc.declare_dram_parameter("b2b", [128, GH], f32, isOutput=False)
    bf1b_d = nc.declare_dram_parameter("bf1b", [128, 2], f32, isOutput=False)
    bf2b_d = nc.declare_dram_parameter("bf2b", [L, 1], f32, isOutput=False)
    out_d = nc.declare_dram_parameter("out", [L, BL], f32, isOutput=True)

    with tile.TileContext(nc) as tc:
        with (
            tc.tile_pool(name="const", bufs=1) as cpool,
            tc.tile_pool(name="megp", bufs=BL) as megpool,
            tc.tile_pool(name="work", bufs=3) as wpool,
            tc.tile_pool(name="psY", bufs=2, space="PSUM") as psY,
            tc.tile_pool(name="psL", bufs=2, space="PSUM") as psL,
            tc.tile_pool(name="psB", bufs=4, space="PSUM") as psB,
        ):
            # fp8 consts ride the ACT ring first (idle at start, and the
            # DMA engines are free before the meg stream ramps); ctB (only
            # needed by the GCN/head stages) is emitted after m1/m3.
            ct8 = cpool.tile([128, HC, GH + 128], fp8)
            nc.scalar.dma_start(ct8[:], ct8_d[:])
            ctB = cpool.tile([128, C_W], bf16)
            b1t = b2t = bf1t = bf2t = None
            if not b1_zero:
                b1t = cpool.tile([128, GH], f32, name="b1t")
                nc.scalar.dma_start(b1t[:], b1b_d[:])
            if not b2_zero:
                b2t = cpool.tile([128, GH], f32, name="b2t")
                nc.scalar.dma_start(b2t[:], b2b_d[:])
            if not bf1_zero:
                bf1t = cpool.tile([128, 2], f32, name="bf1t")
                nc.scalar.dma_start(bf1t[:], bf1b_d[:])
            if not bf2_zero:
                bf2t = cpool.tile([L, 1], f32, name="bf2t")
                nc.scalar.dma_start(bf2t[:], bf2b_d[:])
            catT6 = cpool.tile([128, BL], bf16)
            h1r = cpool.tile([128, 2, BL], bf16)

            # meg delivery: ONE transfer per graph (lhT fp8 + bf16 tail in
            # one byte blob) in consumption order, alternating over the two
            # HWDGE rings (evens on sync, odds on ACT/scalar).  The XBAR
            # transposes ride the sync ring, whose bulk drains before the
            # first XBAR's data is ready; the scalar engine queue stays
            # free for the sigmoids after its 6 early triggers.
            megs8 = []   # [128, HC, S] fp8 lhT views
            megsb = []   # [128, (SC+1)*N] bf16 views (P' chunks + Ahat)
            for g in range(BL):
                m = megpool.tile([128, MEGW8], fp8, tag=f"m{g}", bufs=1,
                                 name=f"m{g}")
                if g % 2 == 1:
                    nc.scalar.dma_start(m[:], meg8_d[g])
                else:
                    nc.sync.dma_start(m[:], meg8_d[g])
                megs8.append(m[:, 0:HC * S].rearrange(
                    "p (c s) -> p c s", c=HC))
                megsb.append(m[:, HC * S:MEGW8].bitcast(bf16))
            nc.scalar.dma_start(ctB[:], consts_d[:])

            W1c = ct8[:, :, 0:GH]          # [128, HC, GH]
            WRBc = ct8[:, :, GH:GH + 128]  # [128, HC, 128]
            W2c = ctB[:, C_W2:C_W2 + GH]
            MEAN = ctB[:, C_MEAN:C_MEAN + 1]
            IDENT = ctB[:, C_IDENT:C_IDENT + 128]

            yT_ps = [None] * BL
            lg_ps = [None] * BL
            gate_sb = [None] * BL
            y_sb = [None] * BL
            t1sb = [None] * BL
            x1 = [None] * BL
            x1t = [None] * BL
            t2sb = [None] * BL
            x2 = [None] * BL

            def relu_to(out_sb, z_ps, bias_tile, tag):
                if bias_tile is None:
                    nc.vector.tensor_scalar_max(out_sb[:], z_ps[:], 0.0)
                else:
                    tmp = wpool.tile([128, GH], f32, tag=tag + "b", bufs=2,
                                     name=tag + "b")
                    nc.vector.tensor_tensor(tmp[:], z_ps[:], bias_tile[:],
                                            ALU.add)
                    nc.vector.tensor_scalar_max(out_sb[:], tmp[:], 0.0)

            ygsb2 = [None] * (BL // 2)
            y2 = [None] * (BL // 2)

            def gate_into_y(g):
                """multiply broadcast sigmoid gates into yT, XBAR per pair."""
                k, half = g // 2, g % 2
                if half == 0:
                    ygsb2[k] = wpool.tile([128, 2, S], bf16, tag="ygsb",
                                          bufs=2, name="ygsb")
                nc.vector.tensor_tensor(ygsb2[k][:, half, :], yT_ps[g][:],
                                        gate_sb[g][:], ALU.mult)
                if half == 1:
                    y2[k] = wpool.tile([128, 2 * SC, 128], bf16, tag="ysb",
                                       bufs=BL // 2, name="y_sb")
                    # all XBAR transposes on the sync ring (its meg bulk
                    # drains early), clearing in data-ready order
                    nc.sync.dma_start(y2[k][:], ygsb2[k][:], transpose=True)
                    y_sb[2 * k] = y2[k][:, 0:SC, :]
                    y_sb[2 * k + 1] = y2[k][:, SC:2 * SC, :]

            def pool_g(g):
                t1_ps = psB.tile([128, GH], f32, tag="mm", name="t1_ps")
                for c in range(SC):
                    nc.tensor.matmul(
                        t1_ps[:],
                        megsb[g][:, c * N:(c + 1) * N],
                        y_sb[g][:, c, :], start=(c == 0), stop=(c == SC - 1))
                t1sb[g] = wpool.tile([128, GH], bf16, tag="t1sb", bufs=BL,
                                     name="t1sb")
                nc.vector.tensor_copy(t1sb[g][:], t1_ps[:])

            def z1_g(g):
                z_ps = psB.tile([128, GH], f32, tag="mm", name="z_ps")
                nc.tensor.matmul(z_ps[:], megsb[g][:, SC * N:(SC + 1) * N],
                                 t1sb[g][:], start=True, stop=True)
                x1[g] = wpool.tile([128, GH], bf16, tag="x1", bufs=BL,
                                   name="x1")
                relu_to(x1[g], z_ps, b1t, "x1")

            def xt_g(g):
                xt_ps = psB.tile([128, GH], bf16, tag="mm", name="xt_ps")
                nc.tensor.transpose(xt_ps[:], x1[g][:], IDENT)
                x1t[g] = wpool.tile([128, GH], bf16, tag="x1t", bufs=BL,
                                    name="x1t")
                nc.vector.tensor_copy(x1t[g][:], xt_ps[:])

            def w2_g(g):
                t2_ps = psB.tile([128, GH], f32, tag="mm", name="t2_ps")
                nc.tensor.matmul(t2_ps[:], x1t[g][:], W2c,
                                 start=True, stop=True)
                t2sb[g] = wpool.tile([128, GH], bf16, tag="t2sb", bufs=BL,
                                     name="t2sb")
                nc.scalar.copy(t2sb[g][:], t2_ps[:])

            def z2_g(g):
                z2_ps = psB.tile([128, GH], f32, tag="mm", name="z2_ps")
                nc.tensor.matmul(z2_ps[:], megsb[g][:, SC * N:(SC + 1) * N],
                                 t2sb[g][:], start=True, stop=True)
                x2[g] = wpool.tile([128, GH], bf16, tag="x2", bufs=BL,
                                   name="x2")
                relu_to(x2[g], z2_ps, b2t, "x2")

            def mp_g(g):
                mp_ps = psB.tile([128, 1], f32, tag="mm", name="mp_ps")
                nc.tensor.matmul(mp_ps[:], x2[g][:], MEAN,
                                 start=True, stop=True)
                nc.vector.tensor_copy(catT6[:, g:g + 1], mp_ps[:])

            # ---- unified software pipeline: projection/gate (steps 0..8)
            # with pooling+GCN stages for finished pairs filling PE slack ----
            STAGES = [pool_g, z1_g, xt_g, w2_g, z2_g, mp_g]
            for s in range(BL + 7):
                # deep stages first (their data has been ready the longest);
                # pool (stage 0) last — its XBAR data is the freshest.
                for si in range(len(STAGES) - 1, 0, -1):
                    k2 = s - 3 - si  # == 2*k when this pair's stage is due
                    if k2 >= 0 and k2 % 2 == 0 and k2 < BL:
                        STAGES[si](k2)
                        STAGES[si](k2 + 1)
                if s < BL:
                    if s >= 1:
                        gate_sb[s - 1] = wpool.tile([128, S], bf16, tag="gt",
                                                    bufs=2, name="gate_sb")
                        nc.scalar.activation(gate_sb[s - 1][:],
                                             lg_ps[s - 1][:], AFT.Sigmoid,
                                             bias=float(br_val),
                                             scale=1.0 / SC_W)
                    yT_ps[s] = psY.tile([128, S], f32, tag="yt", name="yT_ps")
                    lg_ps[s] = psL.tile([128, S], f32, tag="lg", name="lg_ps")
                    for j in range(HC // 2):
                        lht_j = megs8[s][:, 2 * j:2 * j + 2, :]
                        nc.tensor.matmul(
                            yT_ps[s][:], W1c[:, 2 * j:2 * j + 2, :],
                            lht_j, start=(j == 0), stop=(j == HC // 2 - 1),
                            perf_mode=MPM.DoubleRow)
                        nc.tensor.matmul(
                            lg_ps[s][:], WRBc[:, 2 * j:2 * j + 2, :],
                            lht_j, start=(j == 0), stop=(j == HC // 2 - 1),
                            perf_mode=MPM.DoubleRow)
                    if s >= 1:
                        gate_into_y(s - 1)
                if s == BL:
                    gate_sb[BL - 1] = wpool.tile([128, S], bf16, tag="gt",
                                                 bufs=2, name="gate_sb")
                    nc.scalar.activation(gate_sb[BL - 1][:],
                                         lg_ps[BL - 1][:], AFT.Sigmoid,
                                         bias=float(br_val),
                                         scale=1.0 / SC_W)
                    gate_into_y(BL - 1)
                if s >= 3 and (s - 3) % 2 == 0 and s - 3 < BL:
                    pool_g(s - 3)
                    pool_g(s - 2)

            # ---------- FC head over all BL graphs ----------
            h1_ps = []
            for hh in range(2):
                hp = psB.tile([128, BL], f32, tag="mm", name=f"h1_ps{hh}")
                for c in range(FC):
                    lhsT = ctB[:, C_WF1 + (c * 2 + hh) * 128:
                               C_WF1 + (c * 2 + hh + 1) * 128]
                    rhs = (ctB[:, C_CLS + c * BL:C_CLS + (c + 1) * BL]
                           if c < HC else catT6[:])
                    nc.tensor.matmul(hp[:], lhsT, rhs, start=(c == 0),
                                     stop=(c == FC - 1))
                h1_ps.append(hp)
            for hh in range(2):
                if bf1t is None:
                    nc.vector.tensor_scalar_max(h1r[:, hh, :], h1_ps[hh][:],
                                                0.0)
                else:
                    nc.vector.tensor_scalar(h1r[:, hh, :], h1_ps[hh][:],
                                            bf1t[:, hh:hh + 1], 0.0,
                                            ALU.add, ALU.max)
            out_ps = psB.tile([L, BL], f32, tag="mm", name="out_ps")
            for hh in range(2):
                nc.tensor.matmul(out_ps[:],
                                 ctB[:, C_WF2 + hh * L:C_WF2 + (hh + 1) * L],
                                 h1r[:, hh, :], start=(hh == 0),
                                 stop=(hh == 1))
            outs = cpool.tile([L, BL], f32)
            if bf2t is None:
                nc.vector.tensor_copy(outs[:], out_ps[:])
            else:
                nc.vector.tensor_scalar_add(outs[:], out_ps[:], bf2t[:])
            nc.sync.dma_start(out_d[:], outs[:])

    _split_multi_waits(nc)
    return nc


def _prepare_in_maps(inputs):
    lh = np.ascontiguousarray(np.asarray(inputs["last_hidden"], dtype=np.float32))
    submap = np.asarray(inputs["submap"]).astype(np.int64)
    edge_index = np.asarray(inputs["edge_index"]).astype(np.int64)
    assert lh.shape == (B, S, H)
    assert int(inputs.get("num_nodes", N)) == N

    wr = np.asarray(inputs["wr"], dtype=np.float32)
    br = float(np.asarray(inputs["br"], dtype=np.float32))
    W1 = np.asarray(inputs["W1"], dtype=np.float32)
    b1 = np.asarray(inputs["b1"], dtype=np.float32)
    W2 = np.asarray(inputs["W2"], dtype=np.float32)
    b2 = np.asarray(inputs["b2"], dtype=np.float32)
    Wf1 = np.asarray(inputs["Wf1"], dtype=np.float32)
    bf1 = np.asarray(inputs["bf1"], dtype=np.float32)
    Wf2 = np.asarray(inputs["Wf2"], dtype=np.float32)
    bf2 = np.asarray(inputs["bf2"], dtype=np.float32)

    # ---- host-side index prep: adjacency, degrees, counts ----
    src = edge_index[:, 0, :]
    dst = edge_index[:, 1, :]
    flat = (np.arange(B, dtype=np.int64)[:, None] * (N * N) + src * N + dst)
    A = np.bincount(flat.reshape(-1), minlength=B * N * N).astype(np.float32)
    A = A.reshape(B, N, N) + np.eye(N, dtype=np.float32)[None]
    deg = A.sum(axis=1)                      # in-degree incl self-loops
    dinv = 1.0 / np.sqrt(deg)
    ahat = A * dinv[:, :, None] * dinv[:, None, :]

    cflat = np.arange(B, dtype=np.int64)[:, None] * N + submap
    cnt = np.bincount(cflat.reshape(-1), minlength=B * N).astype(np.float32)
    invc = 1.0 / np.maximum(cnt.reshape(B, N), 1.0)

    P = (submap[:, :, None] == np.arange(N)[None, None, :]).astype(np.float32)
    P *= (invc * dinv)[:, None, :] * (1.0 / SC_W)

    # ---- merged mega-tensor: fp8 lhT bytes + bf16 P'/Ahat bytes ----
    lht = np.ascontiguousarray(
        lh.astype(E4M3).reshape(B, S, HC, 128).transpose(0, 3, 2, 1))
    p_r = P.astype(BF16).reshape(B, SC, 128, N).transpose(0, 2, 1, 3)
    megb = np.empty((B, 128, (SC + 1) * N), dtype=BF16)
    megb[:, :, 0:SC * N] = p_r.reshape(B, 128, SC * N)
    megb[:, :, SC * N:(SC + 1) * N] = ahat.astype(BF16)
    meg8 = np.empty((B, 128, MEGW8), dtype=np.uint8)
    meg8[:, :, 0:HC * S] = lht.reshape(B, 128, HC * S).view(np.uint8)
    meg8[:, :, HC * S:] = megb.view(np.uint8)
    meg8 = meg8.view(E4M3)

    # ---- fp8 consts: scaled W1 + broadcast wr ----
    ct8 = np.zeros((128, C8_W), dtype=np.float32)
    ct8[:, C8_W1:C8_W1 + HC * GH] = (
        (SC_W * W1).reshape(HC, 128, GH).transpose(1, 0, 2).reshape(128, -1))
    ct8[:, C8_WRB:C8_WRB + HC * 128] = np.repeat(
        (SC_W * wr).reshape(HC, 128).T, 128, axis=1).reshape(128, HC * 128)
    # interleave into [p, hc, GH+128] layout matching the SBUF tile
    ct8_t = np.empty((128, HC, GH + 128), dtype=E4M3)
    ct8_t[:, :, 0:GH] = ct8[:, 0:HC * GH].reshape(128, HC, GH).astype(E4M3)
    ct8_t[:, :, GH:] = ct8[:, HC * GH:].reshape(128, HC, 128).astype(E4M3)
    ct8_flat = np.ascontiguousarray(ct8_t.reshape(128, -1))

    # ---- bf16 consts, cls block differs per core ----
    consts = np.zeros((128, C_W), dtype=np.float32)
    consts[:, C_W2:C_W2 + GH] = W2
    consts[:, C_WF1:C_WF1 + FC * 2 * 128] = (
        Wf1.reshape(FC, 128, 2, 128).transpose(1, 0, 2, 3).reshape(128, -1))
    consts[:, C_WF2:C_WF2 + 2 * L] = (
        Wf2.reshape(2, 128, L).transpose(1, 0, 2).reshape(128, 2 * L))
    consts[:, C_MEAN] = 1.0 / N
    consts[:, C_IDENT:C_IDENT + 128] = np.eye(128, dtype=np.float32)

    b1b = np.ascontiguousarray(np.broadcast_to(b1, (128, GH)).astype(np.float32))
    b2b = np.ascontiguousarray(np.broadcast_to(b2, (128, GH)).astype(np.float32))
    bf1b = np.ascontiguousarray(bf1.reshape(2, 128).T.astype(np.float32))
    bf2b = np.ascontiguousarray(bf2.reshape(L, 1).astype(np.float32))

    in_maps = []
    for i in range(NCORES):
        sl = slice(i * BL, (i + 1) * BL)
        ci = consts.copy()
        ci[:, C_CLS:C_CLS + HC * BL] = (
            lh[sl, 0, :].reshape(BL, HC, 128).transpose(2, 1, 0)
            .reshape(128, HC * BL))
        cb = ci.astype(BF16)
        in_maps.append({
            "meg8": np.ascontiguousarray(meg8[sl]),
            "ct8": ct8_flat,
            "consts": cb,
            "b1b": b1b, "b2b": b2b, "bf1b": bf1b, "bf2b": bf2b,
        })
    flags = (br, bool(np.all(b1 == 0)), bool(np.all(b2 == 0)),
             bool(np.all(bf1 == 0)), bool(np.all(bf2 == 0)))
    return in_maps, flags


def _run(inputs, trace=False):
    in_maps, flags = _prepare_in_maps(inputs)
    key = ("prog",) + flags
    if key not in _CACHE:
        _CACHE[key] = build_program(*flags)
    nc = _CACHE[key]
    res = run_bass_kernel_spmd(nc, in_maps, list(range(NCORES)), trace=trace)
    out = np.concatenate(
        [np.asarray(res.results[i]["out"]).T for i in range(NCORES)],
        axis=0).astype(np.float32)
    return out, res


def kernel(**inputs) -> np.ndarray:
    out, _ = _run(inputs, trace=False)
    return out
